# revision 37
# baseline (speedup 1.0000x reference)
"""Trainium2 Bass kernel for nn_FCFClient (scatter_memory).

reference(X, global_Y, likes, movie_ids) -> (avg_loss, y_grad):
    s     = X @ global_Y                    # (I,) per-item scores
    err_j = s[id_j] - likes_j               # per interaction
    loss  = mean(err^2)
    y_grad[:, m] = (2*err_win(m)*X.T + 2*lam*Y[:, m]) / N   for touched m (last
                   occurrence wins, CPU-jax scatter semantics), else 0.

All per-interaction work is compressed host-side (pure index manipulation /
bincount over ids+likes) into dense per-column arrays so that
    loss = (sum_m cnt*s^2 - 2*sl*s + sl2) / N
    y_grad[:, m] = c[m]*X.T + t2[m]*Y[:, m]
        c[m]  = s[m]*tch2[m] - wl2[m],  tch2 = touched*2/N, wl2 = win_like*tch2
        t2[m] = touched[m]*2*lam/N

Device (per core, 16384-column shard): stream Y; PE computes s with X as the
stationary operand (1-column weight loads; fp32 matmul is two-pass LOW/HIGH on
this PE so fat weight loads are poison); s bounces through DRAM into a
column-tiled layout for the loss reduction and for c; c returns to row layout
via one PE transpose + pack-DMA; GPSIMD partition_broadcast replicates c
across partitions (t2 arrives host-replicated); DVE applies y = y*t2rep then y = crep*X_col + y (fused
scalar_tensor_tensor). Memory roofline: read Y + write y_grad.
"""

import numpy as np

import concourse.bass as bass
import concourse.tile as tile
from concourse import bacc, mybir, bass_utils
from concourse.masks import make_identity

K = 256
I = 131072
N = 262144
NCORES = 8
M = I // NCORES          # 16384 columns per core
LAMBDA_REG = 0.01
F = 4096                 # columns per chunk
NCH = M // F             # 8 chunks
JT = F // 128            # 16 column-tiles (of 128) per chunk
f32 = mybir.dt.float32
bf16 = mybir.dt.bfloat16

_cache = {}


def _build():
    nc = bacc.Bacc("TRN2", target_bir_lowering=False, debug=False,
                   num_devices=NCORES)
    Yd = nc.dram_tensor("y", (K, M), f32, kind="ExternalInput").ap()
    x2 = nc.dram_tensor("x2", (128, 2), f32, kind="ExternalInput").ap()
    cnt = nc.dram_tensor("cnt", (128, 128), f32, kind="ExternalInput").ap()
    sl = nc.dram_tensor("sl", (128, 128), f32, kind="ExternalInput").ap()
    sl2 = nc.dram_tensor("sl2", (128, 128), f32, kind="ExternalInput").ap()
    wl2 = nc.dram_tensor("wl2", (128, 128), f32, kind="ExternalInput").ap()
    tch2 = nc.dram_tensor("tch2", (128, 128), f32, kind="ExternalInput").ap()
    t2r = nc.dram_tensor("t2r", (128, M), f32, kind="ExternalInput").ap()
    yg = nc.dram_tensor("yg", (K, M), f32, kind="ExternalOutput").ap()
    pl = nc.dram_tensor("pl", (1, 1), f32, kind="ExternalOutput").ap()

    mult = mybir.AluOpType.mult
    add = mybir.AluOpType.add
    sub = mybir.AluOpType.subtract

    with tile.TileContext(nc) as tc:
        with (
            tc.tile_pool(name="ybuf", bufs=2) as ypool,
            tc.tile_pool(name="crepb", bufs=2) as crepool,
            tc.tile_pool(name="ctb", bufs=3) as ctpool,
            tc.tile_pool(name="small", bufs=1) as small,
            tc.tile_pool(name="ps", bufs=4, space="PSUM") as pspool,
            tc.tile_pool(name="dram", bufs=1, space="DRAM") as drampool,
        ):
            x2_sb = small.tile([128, 2], f32, tag="x2")
            nc.sync.dma_start(x2_sb[:], x2[:])
            cnt_sb = small.tile([128, 128], f32, tag="cnt")
            nc.sync.dma_start(cnt_sb[:], cnt[:])
            sl_sb = small.tile([128, 128], f32, tag="sl")
            nc.sync.dma_start(sl_sb[:], sl[:])
            sl2_sb = small.tile([128, 128], f32, tag="sl2")
            nc.sync.dma_start(sl2_sb[:], sl2[:])
            wl2_sb = small.tile([128, 128], f32, tag="wl2")
            nc.sync.dma_start(wl2_sb[:], wl2[:])
            tch2_sb = small.tile([128, 128], f32, tag="tch2")
            nc.sync.dma_start(tch2_sb[:], tch2[:])
            s_sb = small.tile([128, 128], f32, tag="s")
            ident = small.tile([128, 128], f32, tag="id")
            make_identity(nc, ident[:])
            ones = small.tile([128, 1], f32, tag="ones")
            nc.vector.memset(ones[:], 1.0)
            s_dram = drampool.tile([1, M], f32, tag="sd")

            def issue_t2rep(qq):
                # t2 arrives host-replicated across partitions: plain
                # contiguous load, no on-chip replication, no dep chain
                tsl = slice(qq * F, (qq + 1) * F)
                rep = crepool.tile([128, F], f32, tag="t2rep")
                nc.scalar.dma_start(rep[:], t2r[:, tsl])
                return rep

            def produce(q):
                # load y chunk, compute s, derive c, replicate c/t2 across
                # partitions. Returns tiles consumed LAG iterations later.
                msl = slice(q * F, (q + 1) * F)
                y0 = ypool.tile([128, F], f32, tag="y0")
                y1 = ypool.tile([128, F], f32, tag="y1")
                nc.scalar.dma_start(y0[:], Yd[0:128, msl])
                nc.scalar.dma_start(y1[:], Yd[128:256, msl])
                t2rep = issue_t2rep(q)

                # s row chunk: X stationary (1-col weights), Y moving
                srow = ctpool.tile([1, F], f32, tag="rowbuf")
                for b in range(F // 512):
                    bsl = slice(b * 512, (b + 1) * 512)
                    sp = pspool.tile([1, 512], f32, tag="sp")
                    nc.tensor.matmul(sp[:], lhsT=x2_sb[:, 0:1],
                                     rhs=y0[:, bsl], start=True, stop=False)
                    nc.tensor.matmul(sp[:], lhsT=x2_sb[:, 1:2],
                                     rhs=y1[:, bsl], start=False, stop=True)
                    nc.scalar.copy(srow[0:1, bsl], sp[:])

                # bounce s chunk through DRAM into column-tiled layout
                nc.sync.dma_start(s_dram[0:1, msl], srow[:])
                csl = slice(q * JT, (q + 1) * JT)
                stl = ctpool.tile([128, JT], f32, tag="stl")
                nc.sync.dma_start(
                    stl[:],
                    s_dram[0:1, msl].rearrange("o (j p) -> (o p) j", p=128))
                # persistent copy for the end-of-kernel loss reduction
                nc.scalar.copy(s_sb[:, csl], stl[:])

                # c = s*tch2 - wl2 (column-tiled)
                ctl = ctpool.tile([128, JT], f32, tag="ctl")
                nc.vector.tensor_tensor(out=ctl[:], in0=stl[:],
                                        in1=tch2_sb[:, csl], op=mult)
                nc.vector.tensor_tensor(out=ctl[:], in0=ctl[:],
                                        in1=wl2_sb[:, csl], op=sub)

                # c tiled -> row fragment
                ctp = pspool.tile([JT, 128], f32, tag="ctp")
                nc.tensor.transpose(out=ctp[:], in_=ctl[:],
                                    identity=ident[:])
                ctsb = ctpool.tile([JT, 128], f32, tag="ctsb")
                nc.scalar.copy(ctsb[:], ctp[:])
                crow = ctpool.tile([1, F], f32, tag="rowbuf")
                nc.sync.dma_start(crow[:], ctsb[:])

                # replicate c chunk across partitions (gpsimd)
                crep = crepool.tile([128, F], f32, tag="crep")
                nc.gpsimd.partition_broadcast(crep[:], crow[:])
                return y0, y1, t2rep, crep

            def consume(q, y0, y1, t2rep, crep):
                # y = y*t2rep ; y = crep*X_col + y ; store
                msl = slice(q * F, (q + 1) * F)
                for h, ytile in ((0, y0), (1, y1)):
                    nc.vector.tensor_tensor(out=ytile[:], in0=ytile[:],
                                            in1=t2rep[:], op=mult)
                    nc.vector.scalar_tensor_tensor(
                        out=ytile[:], in0=crep[:],
                        scalar=x2_sb[:, h:h + 1], in1=ytile[:],
                        op0=mult, op1=add)
                nc.sync.dma_start(yg[0:128, msl], y0[:])
                nc.sync.dma_start(yg[128:256, msl], y1[:])

            LAG = 1
            state = {}
            for q in range(NCH + LAG):
                if q < NCH:
                    state[q] = produce(q)
                if q >= LAG:
                    consume(q - LAG, *state.pop(q - LAG))

            # loss partial: sum(cnt*s^2 - 2*sl*s + sl2) over this shard
            lt = small.tile([128, 128], f32, tag="lt")
            a1 = small.tile([128, 1], f32, tag="a1")
            a2 = small.tile([128, 1], f32, tag="a2")
            a3 = small.tile([128, 1], f32, tag="a3")
            nc.vector.tensor_tensor(out=lt[:], in0=cnt_sb[:], in1=s_sb[:],
                                    op=mult)
            nc.vector.tensor_tensor(out=lt[:], in0=lt[:], in1=s_sb[:],
                                    op=mult)
            nc.vector.tensor_reduce(out=a1[:], in_=lt[:],
                                    axis=mybir.AxisListType.X, op=add)
            nc.vector.tensor_tensor(out=lt[:], in0=sl_sb[:], in1=s_sb[:],
                                    op=mult)
            nc.vector.tensor_reduce(out=a2[:], in_=lt[:],
                                    axis=mybir.AxisListType.X, op=add)
            nc.vector.tensor_reduce(out=a3[:], in_=sl2_sb[:],
                                    axis=mybir.AxisListType.X, op=add)
            nc.vector.tensor_scalar_mul(a2[:], a2[:], -2.0)
            nc.vector.tensor_add(out=a1[:], in0=a1[:], in1=a2[:])
            nc.vector.tensor_add(out=a1[:], in0=a1[:], in1=a3[:])
            plp = pspool.tile([1, 1], f32, tag="sp")
            nc.tensor.matmul(plp[:], lhsT=a1[:], rhs=ones[:], start=True,
                             stop=True)
            plsb = small.tile([1, 1], f32, tag="plsb")
            nc.scalar.copy(plsb[:], plp[:])
            nc.sync.dma_start(pl[:], plsb[:])

    nc.compile()
    return nc


def _prep_inputs(X, global_Y, likes, movie_ids):
    X = np.asarray(X, np.float32)
    Y = np.asarray(global_Y, np.float32)
    likes = np.asarray(likes, np.float32)
    ids = np.asarray(movie_ids).astype(np.int64)

    cnt = np.bincount(ids, minlength=I).astype(np.float32)
    likes64 = likes.astype(np.float64)
    sl = np.bincount(ids, weights=likes64, minlength=I).astype(np.float32)
    sl2 = np.bincount(ids, weights=likes64 * likes64,
                      minlength=I).astype(np.float32)
    touched = cnt > 0
    lidx = np.zeros(I, np.int64)
    lidx[ids] = np.arange(N)          # last occurrence wins
    win_like = np.where(touched, likes[lidx], 0.0).astype(np.float32)
    tch2 = (touched * (2.0 / N)).astype(np.float32)
    wl2 = (win_like.astype(np.float64) * tch2).astype(np.float32)
    t2 = (touched * (2.0 * LAMBDA_REG / N)).astype(np.float32)

    x2 = np.ascontiguousarray(X.reshape(2, 128).T)

    def pack(a):
        # local column m = j*128 + p  ->  [p, j]
        return np.ascontiguousarray(a.reshape(128, 128).T)

    in_maps = []
    for d in range(NCORES):
        sl_d = slice(d * M, (d + 1) * M)
        in_maps.append({
            "y": np.ascontiguousarray(Y[:, sl_d]),
            "x2": x2,
            "cnt": pack(cnt[sl_d]),
            "sl": pack(sl[sl_d]),
            "sl2": pack(sl2[sl_d]),
            "wl2": pack(wl2[sl_d]),
            "tch2": pack(tch2[sl_d]),
            "t2r": np.ascontiguousarray(np.broadcast_to(t2[sl_d], (128, M))),
        })
    return in_maps


def kernel_run(X, global_Y, likes, movie_ids, trace=False, tmpdir=None):
    if "nc" not in _cache:
        _cache["nc"] = _build()
    nc = _cache["nc"]
    in_maps = _prep_inputs(X, global_Y, likes, movie_ids)
    res = bass_utils.run_bass_kernel_spmd(
        nc, in_maps, core_ids=list(range(NCORES)), trace=trace, tmpdir=tmpdir)
    y_grad = np.concatenate([res.results[d]["yg"] for d in range(NCORES)],
                            axis=1)
    loss = np.float32(
        sum(float(res.results[d]["pl"][0, 0]) for d in range(NCORES)) / N)
    return (loss, y_grad), res


def kernel(X, global_Y, likes, movie_ids):
    out, _ = kernel_run(X, global_Y, likes, movie_ids, trace=False)
    return out


# revision 38
# speedup vs baseline: 1.1104x; 1.1104x over previous
"""Trainium2 Bass kernel for nn_FCFClient (scatter_memory).

reference(X, global_Y, likes, movie_ids) -> (avg_loss, y_grad):
    s     = X @ global_Y                    # (I,) per-item scores
    err_j = s[id_j] - likes_j               # per interaction
    loss  = mean(err^2)
    y_grad[:, m] = (2*err_win(m)*X.T + 2*lam*Y[:, m]) / N   for touched m (last
                   occurrence wins, CPU-jax scatter semantics), else 0.

All per-interaction work is compressed host-side (pure index manipulation /
bincount over ids+likes) into dense per-column arrays so that
    loss = (sum_m cnt*s^2 - 2*sl*s + sl2) / N
    y_grad[:, m] = c[m]*X.T + t2[m]*Y[:, m]
        c[m]  = s[m]*tch2[m] - wl2[m],  tch2 = touched*2/N, wl2 = win_like*tch2
        t2[m] = touched[m]*2*lam/N

Device (per core, 16384-column shard): stream Y; PE computes s with X as the
stationary operand (1-column weight loads; fp32 matmul is two-pass LOW/HIGH on
this PE so fat weight loads are poison); s bounces through DRAM into a
column-tiled layout for the loss reduction and for c; c returns to row layout
via one PE transpose + pack-DMA; GPSIMD partition_broadcast replicates c
across partitions (t2 arrives host-replicated); DVE applies y = y*t2rep then y = crep*X_col + y (fused
scalar_tensor_tensor). Memory roofline: read Y + write y_grad.
"""

import numpy as np

import concourse.bass as bass
import concourse.tile as tile
from concourse import bacc, mybir, bass_utils
from concourse.masks import make_identity

K = 256
I = 131072
N = 262144
NCORES = 8
M = I // NCORES          # 16384 columns per core
LAMBDA_REG = 0.01
F = 2048                 # columns per chunk
NCH = M // F             # 8 chunks
JT = F // 128            # 16 column-tiles (of 128) per chunk
f32 = mybir.dt.float32
bf16 = mybir.dt.bfloat16

_cache = {}


def _build():
    nc = bacc.Bacc("TRN2", target_bir_lowering=False, debug=False,
                   num_devices=NCORES)
    Yd = nc.dram_tensor("y", (K, M), f32, kind="ExternalInput").ap()
    x2 = nc.dram_tensor("x2", (128, 2), f32, kind="ExternalInput").ap()
    cnt = nc.dram_tensor("cnt", (128, 128), f32, kind="ExternalInput").ap()
    sl = nc.dram_tensor("sl", (128, 128), f32, kind="ExternalInput").ap()
    sl2 = nc.dram_tensor("sl2", (128, 128), f32, kind="ExternalInput").ap()
    wl2 = nc.dram_tensor("wl2", (128, 128), f32, kind="ExternalInput").ap()
    tch2 = nc.dram_tensor("tch2", (128, 128), f32, kind="ExternalInput").ap()
    t2r = nc.dram_tensor("t2r", (128, M), f32, kind="ExternalInput").ap()
    yg = nc.dram_tensor("yg", (K, M), f32, kind="ExternalOutput").ap()
    pl = nc.dram_tensor("pl", (1, 1), f32, kind="ExternalOutput").ap()

    mult = mybir.AluOpType.mult
    add = mybir.AluOpType.add
    sub = mybir.AluOpType.subtract

    with tile.TileContext(nc) as tc:
        with (
            tc.tile_pool(name="ybuf", bufs=4) as ypool,
            tc.tile_pool(name="crepb", bufs=4) as crepool,
            tc.tile_pool(name="ctb", bufs=2) as ctpool,
            tc.tile_pool(name="small", bufs=1) as small,
            tc.tile_pool(name="ps", bufs=4, space="PSUM") as pspool,
            tc.tile_pool(name="dram", bufs=1, space="DRAM") as drampool,
        ):
            x2_sb = small.tile([128, 2], f32, tag="x2")
            nc.sync.dma_start(x2_sb[:], x2[:])
            cnt_sb = small.tile([128, 128], f32, tag="cnt")
            nc.sync.dma_start(cnt_sb[:], cnt[:])
            sl_sb = small.tile([128, 128], f32, tag="sl")
            nc.sync.dma_start(sl_sb[:], sl[:])
            sl2_sb = small.tile([128, 128], f32, tag="sl2")
            nc.sync.dma_start(sl2_sb[:], sl2[:])
            wl2_sb = small.tile([128, 128], f32, tag="wl2")
            nc.sync.dma_start(wl2_sb[:], wl2[:])
            tch2_sb = small.tile([128, 128], f32, tag="tch2")
            nc.sync.dma_start(tch2_sb[:], tch2[:])
            s_sb = small.tile([128, 128], f32, tag="s")
            ident = small.tile([128, 128], f32, tag="id")
            make_identity(nc, ident[:])
            ones = small.tile([128, 1], f32, tag="ones")
            nc.vector.memset(ones[:], 1.0)
            s_dram = drampool.tile([1, M], f32, tag="sd")

            def issue_t2rep(qq):
                # t2 arrives host-replicated across partitions: plain
                # contiguous load, no on-chip replication, no dep chain
                tsl = slice(qq * F, (qq + 1) * F)
                rep = crepool.tile([128, F], f32, tag="t2rep")
                nc.scalar.dma_start(rep[:], t2r[:, tsl])
                return rep

            def produce(q):
                # load y chunk, compute s, derive c, replicate c/t2 across
                # partitions. Returns tiles consumed LAG iterations later.
                msl = slice(q * F, (q + 1) * F)
                y0 = ypool.tile([128, F], f32, tag="y0")
                y1 = ypool.tile([128, F], f32, tag="y1")
                nc.scalar.dma_start(y0[:], Yd[0:128, msl])
                nc.scalar.dma_start(y1[:], Yd[128:256, msl])
                t2rep = issue_t2rep(q)

                # s row chunk: X stationary (1-col weights), Y moving
                srow = ctpool.tile([1, F], f32, tag="srow")
                for b in range(F // 512):
                    bsl = slice(b * 512, (b + 1) * 512)
                    sp = pspool.tile([1, 512], f32, tag="sp")
                    nc.tensor.matmul(sp[:], lhsT=x2_sb[:, 0:1],
                                     rhs=y0[:, bsl], start=True, stop=False)
                    nc.tensor.matmul(sp[:], lhsT=x2_sb[:, 1:2],
                                     rhs=y1[:, bsl], start=False, stop=True)
                    nc.scalar.copy(srow[0:1, bsl], sp[:])

                # bounce s chunk through DRAM into column-tiled layout
                nc.sync.dma_start(s_dram[0:1, msl], srow[:])
                csl = slice(q * JT, (q + 1) * JT)
                stl = ctpool.tile([128, JT], f32, tag="stl")
                nc.sync.dma_start(
                    stl[:],
                    s_dram[0:1, msl].rearrange("o (j p) -> (o p) j", p=128))
                # persistent copy for the end-of-kernel loss reduction
                nc.scalar.copy(s_sb[:, csl], stl[:])

                # c = s*tch2 - wl2 (column-tiled)
                ctl = ctpool.tile([128, JT], f32, tag="ctl")
                nc.vector.tensor_tensor(out=ctl[:], in0=stl[:],
                                        in1=tch2_sb[:, csl], op=mult)
                nc.vector.tensor_tensor(out=ctl[:], in0=ctl[:],
                                        in1=wl2_sb[:, csl], op=sub)

                # c tiled -> row fragment
                ctp = pspool.tile([JT, 128], f32, tag="ctp")
                nc.tensor.transpose(out=ctp[:], in_=ctl[:],
                                    identity=ident[:])
                ctsb = ctpool.tile([JT, 128], f32, tag="ctsb")
                nc.scalar.copy(ctsb[:], ctp[:])
                crow = ctpool.tile([1, F], f32, tag="crow")
                nc.sync.dma_start(crow[:], ctsb[:])

                # replicate c chunk across partitions (gpsimd)
                crep = crepool.tile([128, F], f32, tag="crep")
                nc.gpsimd.partition_broadcast(crep[:], crow[:])
                return y0, y1, t2rep, crep

            def consume(q, y0, y1, t2rep, crep):
                # y = y*t2rep ; y = crep*X_col + y ; store
                msl = slice(q * F, (q + 1) * F)
                for h, ytile in ((0, y0), (1, y1)):
                    nc.vector.tensor_tensor(out=ytile[:], in0=ytile[:],
                                            in1=t2rep[:], op=mult)
                    nc.vector.scalar_tensor_tensor(
                        out=ytile[:], in0=crep[:],
                        scalar=x2_sb[:, h:h + 1], in1=ytile[:],
                        op0=mult, op1=add)
                nc.sync.dma_start(yg[0:128, msl], y0[:])
                nc.sync.dma_start(yg[128:256, msl], y1[:])

            LAG = 2
            state = {}
            for q in range(NCH + LAG):
                if q < NCH:
                    state[q] = produce(q)
                if q >= LAG:
                    consume(q - LAG, *state.pop(q - LAG))

            # loss partial: sum(cnt*s^2 - 2*sl*s + sl2) over this shard
            lt = small.tile([128, 128], f32, tag="lt")
            a1 = small.tile([128, 1], f32, tag="a1")
            a2 = small.tile([128, 1], f32, tag="a2")
            a3 = small.tile([128, 1], f32, tag="a3")
            nc.vector.tensor_tensor(out=lt[:], in0=cnt_sb[:], in1=s_sb[:],
                                    op=mult)
            nc.vector.tensor_tensor(out=lt[:], in0=lt[:], in1=s_sb[:],
                                    op=mult)
            nc.vector.tensor_reduce(out=a1[:], in_=lt[:],
                                    axis=mybir.AxisListType.X, op=add)
            nc.vector.tensor_tensor(out=lt[:], in0=sl_sb[:], in1=s_sb[:],
                                    op=mult)
            nc.vector.tensor_reduce(out=a2[:], in_=lt[:],
                                    axis=mybir.AxisListType.X, op=add)
            nc.vector.tensor_reduce(out=a3[:], in_=sl2_sb[:],
                                    axis=mybir.AxisListType.X, op=add)
            nc.vector.tensor_scalar_mul(a2[:], a2[:], -2.0)
            nc.vector.tensor_add(out=a1[:], in0=a1[:], in1=a2[:])
            nc.vector.tensor_add(out=a1[:], in0=a1[:], in1=a3[:])
            plp = pspool.tile([1, 1], f32, tag="sp")
            nc.tensor.matmul(plp[:], lhsT=a1[:], rhs=ones[:], start=True,
                             stop=True)
            plsb = small.tile([1, 1], f32, tag="plsb")
            nc.scalar.copy(plsb[:], plp[:])
            nc.sync.dma_start(pl[:], plsb[:])

    nc.compile()
    return nc


def _prep_inputs(X, global_Y, likes, movie_ids):
    X = np.asarray(X, np.float32)
    Y = np.asarray(global_Y, np.float32)
    likes = np.asarray(likes, np.float32)
    ids = np.asarray(movie_ids).astype(np.int64)

    cnt = np.bincount(ids, minlength=I).astype(np.float32)
    likes64 = likes.astype(np.float64)
    sl = np.bincount(ids, weights=likes64, minlength=I).astype(np.float32)
    sl2 = np.bincount(ids, weights=likes64 * likes64,
                      minlength=I).astype(np.float32)
    touched = cnt > 0
    lidx = np.zeros(I, np.int64)
    lidx[ids] = np.arange(N)          # last occurrence wins
    win_like = np.where(touched, likes[lidx], 0.0).astype(np.float32)
    tch2 = (touched * (2.0 / N)).astype(np.float32)
    wl2 = (win_like.astype(np.float64) * tch2).astype(np.float32)
    t2 = (touched * (2.0 * LAMBDA_REG / N)).astype(np.float32)

    x2 = np.ascontiguousarray(X.reshape(2, 128).T)

    def pack(a):
        # local column m = j*128 + p  ->  [p, j]
        return np.ascontiguousarray(a.reshape(128, 128).T)

    in_maps = []
    for d in range(NCORES):
        sl_d = slice(d * M, (d + 1) * M)
        in_maps.append({
            "y": np.ascontiguousarray(Y[:, sl_d]),
            "x2": x2,
            "cnt": pack(cnt[sl_d]),
            "sl": pack(sl[sl_d]),
            "sl2": pack(sl2[sl_d]),
            "wl2": pack(wl2[sl_d]),
            "tch2": pack(tch2[sl_d]),
            "t2r": np.ascontiguousarray(np.broadcast_to(t2[sl_d], (128, M))),
        })
    return in_maps


def kernel_run(X, global_Y, likes, movie_ids, trace=False, tmpdir=None):
    if "nc" not in _cache:
        _cache["nc"] = _build()
    nc = _cache["nc"]
    in_maps = _prep_inputs(X, global_Y, likes, movie_ids)
    res = bass_utils.run_bass_kernel_spmd(
        nc, in_maps, core_ids=list(range(NCORES)), trace=trace, tmpdir=tmpdir)
    y_grad = np.concatenate([res.results[d]["yg"] for d in range(NCORES)],
                            axis=1)
    loss = np.float32(
        sum(float(res.results[d]["pl"][0, 0]) for d in range(NCORES)) / N)
    return (loss, y_grad), res


def kernel(X, global_Y, likes, movie_ids):
    out, _ = kernel_run(X, global_Y, likes, movie_ids, trace=False)
    return out


# revision 39
# speedup vs baseline: 1.2000x; 1.0807x over previous
"""Trainium2 Bass kernel for nn_FCFClient (scatter_memory).

reference(X, global_Y, likes, movie_ids) -> (avg_loss, y_grad):
    s     = X @ global_Y                    # (I,) per-item scores
    err_j = s[id_j] - likes_j               # per interaction
    loss  = mean(err^2)
    y_grad[:, m] = (2*err_win(m)*X.T + 2*lam*Y[:, m]) / N   for touched m (last
                   occurrence wins, CPU-jax scatter semantics), else 0.

All per-interaction work is compressed host-side (pure index manipulation /
bincount over ids+likes) into dense per-column arrays so that
    loss = (sum_m cnt*s^2 - 2*sl*s + sl2) / N
    y_grad[:, m] = c[m]*X.T + t2[m]*Y[:, m]
        c[m]  = s[m]*tch2[m] - wl2[m],  tch2 = touched*2/N, wl2 = win_like*tch2
        t2[m] = touched[m]*2*lam/N

Device (per core, 16384-column shard): stream Y; PE computes s with X as the
stationary operand (1-column weight loads; fp32 matmul is two-pass LOW/HIGH on
this PE so fat weight loads are poison); s bounces through DRAM into a
column-tiled layout for the loss reduction and for c; c returns to row layout
via one PE transpose + pack-DMA; GPSIMD partition_broadcast replicates c
across partitions (t2 arrives host-replicated); DVE applies y = y*t2rep then y = crep*X_col + y (fused
scalar_tensor_tensor). Memory roofline: read Y + write y_grad.
"""

import numpy as np

import concourse.bass as bass
import concourse.tile as tile
from concourse import bacc, mybir, bass_utils
from concourse.masks import make_identity

K = 256
I = 131072
N = 262144
NCORES = 8
M = I // NCORES          # 16384 columns per core
LAMBDA_REG = 0.01
F = 2048                 # columns per chunk
NCH = M // F             # 8 chunks
JT = F // 128            # 16 column-tiles (of 128) per chunk
f32 = mybir.dt.float32
bf16 = mybir.dt.bfloat16

_cache = {}


def _build():
    nc = bacc.Bacc("TRN2", target_bir_lowering=False, debug=False,
                   num_devices=NCORES)
    Yd = nc.dram_tensor("y", (K, M), f32, kind="ExternalInput").ap()
    x2 = nc.dram_tensor("x2", (128, 2), f32, kind="ExternalInput").ap()
    cnt = nc.dram_tensor("cnt", (128, 128), f32, kind="ExternalInput").ap()
    sl = nc.dram_tensor("sl", (128, 128), f32, kind="ExternalInput").ap()
    sl2 = nc.dram_tensor("sl2", (128, 128), f32, kind="ExternalInput").ap()
    wl2 = nc.dram_tensor("wl2", (128, 128), f32, kind="ExternalInput").ap()
    tch2 = nc.dram_tensor("tch2", (128, 128), f32, kind="ExternalInput").ap()
    t2r = nc.dram_tensor("t2r", (128, M), f32, kind="ExternalInput").ap()
    yg = nc.dram_tensor("yg", (K, M), f32, kind="ExternalOutput").ap()
    pl = nc.dram_tensor("pl", (1, 1), f32, kind="ExternalOutput").ap()

    mult = mybir.AluOpType.mult
    add = mybir.AluOpType.add
    sub = mybir.AluOpType.subtract

    with tile.TileContext(nc) as tc:
        with (
            tc.tile_pool(name="ybuf", bufs=4) as ypool,
            tc.tile_pool(name="crepb", bufs=4) as crepool,
            tc.tile_pool(name="ctb", bufs=2) as ctpool,
            tc.tile_pool(name="small", bufs=1) as small,
            tc.tile_pool(name="ps", bufs=4, space="PSUM") as pspool,
            tc.tile_pool(name="dram", bufs=1, space="DRAM") as drampool,
        ):
            x2_sb = small.tile([128, 2], f32, tag="x2")
            nc.sync.dma_start(x2_sb[:], x2[:])
            cnt_sb = small.tile([128, 128], f32, tag="cnt")
            nc.sync.dma_start(cnt_sb[:], cnt[:])
            sl_sb = small.tile([128, 128], f32, tag="sl")
            nc.sync.dma_start(sl_sb[:], sl[:])
            sl2_sb = small.tile([128, 128], f32, tag="sl2")
            nc.sync.dma_start(sl2_sb[:], sl2[:])
            wl2_sb = small.tile([128, 128], f32, tag="wl2")
            nc.sync.dma_start(wl2_sb[:], wl2[:])
            tch2_sb = small.tile([128, 128], f32, tag="tch2")
            nc.sync.dma_start(tch2_sb[:], tch2[:])
            s_sb = small.tile([128, 128], f32, tag="s")
            ident = small.tile([128, 128], f32, tag="id")
            make_identity(nc, ident[:])
            ones = small.tile([128, 1], f32, tag="ones")
            nc.vector.memset(ones[:], 1.0)
            s_dram = drampool.tile([1, M], f32, tag="sd")

            def issue_t2rep(qq):
                # t2 arrives host-replicated across partitions: plain
                # contiguous load, no on-chip replication, no dep chain
                tsl = slice(qq * F, (qq + 1) * F)
                rep = crepool.tile([128, F], f32, tag="t2rep")
                nc.scalar.dma_start(rep[:], t2r[:, tsl])
                return rep

            def produce(q):
                # load y chunk, compute s, derive c, replicate c/t2 across
                # partitions. Returns tiles consumed LAG iterations later.
                msl = slice(q * F, (q + 1) * F)
                y0 = ypool.tile([128, F], f32, tag="y0")
                y1 = ypool.tile([128, F], f32, tag="y1")
                nc.scalar.dma_start(y0[:], Yd[0:128, msl])
                nc.scalar.dma_start(y1[:], Yd[128:256, msl])
                t2rep = issue_t2rep(q)

                # s row chunk: X stationary (1-col weights), Y moving
                srow = ctpool.tile([1, F], f32, tag="srow")
                for b in range(F // 512):
                    bsl = slice(b * 512, (b + 1) * 512)
                    sp = pspool.tile([1, 512], f32, tag="sp")
                    nc.tensor.matmul(sp[:], lhsT=x2_sb[:, 0:1],
                                     rhs=y0[:, bsl], start=True, stop=False)
                    nc.tensor.matmul(sp[:], lhsT=x2_sb[:, 1:2],
                                     rhs=y1[:, bsl], start=False, stop=True)
                    nc.scalar.copy(srow[0:1, bsl], sp[:])

                # bounce s chunk through DRAM into column-tiled layout
                nc.sync.dma_start(s_dram[0:1, msl], srow[:])
                csl = slice(q * JT, (q + 1) * JT)
                stl = ctpool.tile([128, JT], f32, tag="stl")
                nc.sync.dma_start(
                    stl[:],
                    s_dram[0:1, msl].rearrange("o (j p) -> (o p) j", p=128))
                # persistent copy for the end-of-kernel loss reduction
                nc.scalar.copy(s_sb[:, csl], stl[:])

                # c = s*tch2 - wl2 (column-tiled)
                ctl = ctpool.tile([128, JT], f32, tag="ctl")
                nc.vector.tensor_tensor(out=ctl[:], in0=stl[:],
                                        in1=tch2_sb[:, csl], op=mult)
                nc.vector.tensor_tensor(out=ctl[:], in0=ctl[:],
                                        in1=wl2_sb[:, csl], op=sub)

                # c tiled -> row fragment
                ctp = pspool.tile([JT, 128], f32, tag="ctp")
                nc.tensor.transpose(out=ctp[:], in_=ctl[:],
                                    identity=ident[:])
                ctsb = ctpool.tile([JT, 128], f32, tag="ctsb")
                nc.scalar.copy(ctsb[:], ctp[:])
                crow = ctpool.tile([1, F], f32, tag="crow")
                nc.sync.dma_start(crow[:], ctsb[:])

                # replicate c chunk across partitions (gpsimd)
                crep = crepool.tile([128, F], f32, tag="crep")
                nc.gpsimd.partition_broadcast(crep[:], crow[:])
                return y0, y1, t2rep, crep

            def consume(q, y0, y1, t2rep, crep):
                # y = y*t2rep ; y = crep*X_col + y ; store
                msl = slice(q * F, (q + 1) * F)
                for h, ytile in ((0, y0), (1, y1)):
                    nc.vector.tensor_tensor(out=ytile[:], in0=ytile[:],
                                            in1=t2rep[:], op=mult)
                    nc.vector.scalar_tensor_tensor(
                        out=ytile[:], in0=crep[:],
                        scalar=x2_sb[:, h:h + 1], in1=ytile[:],
                        op0=mult, op1=add)
                nc.sync.dma_start(yg[0:128, msl], y0[:])
                nc.sync.dma_start(yg[128:256, msl], y1[:])

            LAG = 3
            state = {}
            for q in range(NCH + LAG):
                if q < NCH:
                    state[q] = produce(q)
                if q >= LAG:
                    consume(q - LAG, *state.pop(q - LAG))

            # loss partial: sum(cnt*s^2 - 2*sl*s + sl2) over this shard
            lt = small.tile([128, 128], f32, tag="lt")
            a1 = small.tile([128, 1], f32, tag="a1")
            a2 = small.tile([128, 1], f32, tag="a2")
            a3 = small.tile([128, 1], f32, tag="a3")
            nc.vector.tensor_tensor(out=lt[:], in0=cnt_sb[:], in1=s_sb[:],
                                    op=mult)
            nc.vector.tensor_tensor(out=lt[:], in0=lt[:], in1=s_sb[:],
                                    op=mult)
            nc.vector.tensor_reduce(out=a1[:], in_=lt[:],
                                    axis=mybir.AxisListType.X, op=add)
            nc.vector.tensor_tensor(out=lt[:], in0=sl_sb[:], in1=s_sb[:],
                                    op=mult)
            nc.vector.tensor_reduce(out=a2[:], in_=lt[:],
                                    axis=mybir.AxisListType.X, op=add)
            nc.vector.tensor_reduce(out=a3[:], in_=sl2_sb[:],
                                    axis=mybir.AxisListType.X, op=add)
            nc.vector.tensor_scalar_mul(a2[:], a2[:], -2.0)
            nc.vector.tensor_add(out=a1[:], in0=a1[:], in1=a2[:])
            nc.vector.tensor_add(out=a1[:], in0=a1[:], in1=a3[:])
            plp = pspool.tile([1, 1], f32, tag="sp")
            nc.tensor.matmul(plp[:], lhsT=a1[:], rhs=ones[:], start=True,
                             stop=True)
            plsb = small.tile([1, 1], f32, tag="plsb")
            nc.scalar.copy(plsb[:], plp[:])
            nc.sync.dma_start(pl[:], plsb[:])

    nc.compile()
    return nc


def _prep_inputs(X, global_Y, likes, movie_ids):
    X = np.asarray(X, np.float32)
    Y = np.asarray(global_Y, np.float32)
    likes = np.asarray(likes, np.float32)
    ids = np.asarray(movie_ids).astype(np.int64)

    cnt = np.bincount(ids, minlength=I).astype(np.float32)
    likes64 = likes.astype(np.float64)
    sl = np.bincount(ids, weights=likes64, minlength=I).astype(np.float32)
    sl2 = np.bincount(ids, weights=likes64 * likes64,
                      minlength=I).astype(np.float32)
    touched = cnt > 0
    lidx = np.zeros(I, np.int64)
    lidx[ids] = np.arange(N)          # last occurrence wins
    win_like = np.where(touched, likes[lidx], 0.0).astype(np.float32)
    tch2 = (touched * (2.0 / N)).astype(np.float32)
    wl2 = (win_like.astype(np.float64) * tch2).astype(np.float32)
    t2 = (touched * (2.0 * LAMBDA_REG / N)).astype(np.float32)

    x2 = np.ascontiguousarray(X.reshape(2, 128).T)

    def pack(a):
        # local column m = j*128 + p  ->  [p, j]
        return np.ascontiguousarray(a.reshape(128, 128).T)

    in_maps = []
    for d in range(NCORES):
        sl_d = slice(d * M, (d + 1) * M)
        in_maps.append({
            "y": np.ascontiguousarray(Y[:, sl_d]),
            "x2": x2,
            "cnt": pack(cnt[sl_d]),
            "sl": pack(sl[sl_d]),
            "sl2": pack(sl2[sl_d]),
            "wl2": pack(wl2[sl_d]),
            "tch2": pack(tch2[sl_d]),
            "t2r": np.ascontiguousarray(np.broadcast_to(t2[sl_d], (128, M))),
        })
    return in_maps


def kernel_run(X, global_Y, likes, movie_ids, trace=False, tmpdir=None):
    if "nc" not in _cache:
        _cache["nc"] = _build()
    nc = _cache["nc"]
    in_maps = _prep_inputs(X, global_Y, likes, movie_ids)
    res = bass_utils.run_bass_kernel_spmd(
        nc, in_maps, core_ids=list(range(NCORES)), trace=trace, tmpdir=tmpdir)
    y_grad = np.concatenate([res.results[d]["yg"] for d in range(NCORES)],
                            axis=1)
    loss = np.float32(
        sum(float(res.results[d]["pl"][0, 0]) for d in range(NCORES)) / N)
    return (loss, y_grad), res


def kernel(X, global_Y, likes, movie_ids):
    out, _ = kernel_run(X, global_Y, likes, movie_ids, trace=False)
    return out
